# revision 23
# baseline (speedup 1.0000x reference)
"""AIMNet2 interaction module on 8 TRN2 NeuronCores.

Strategy: the reference gathers per-pair features with idx_j and
segment-sums with the SAME idx_j.  Within the segment of atom n every
gathered row equals the per-atom value, so the pairwise work collapses:

  radial_emb[n]  = E[n] * (segsum(gs)[n] @ W_gs.T)
  radial_q[n]    = (segsum(q*gs)[n] @ W_gs.T)          (q folded host-side)
  avf_sum[n,h,d] = sum_g (E @ AGH)[n,g,h] * segsum(gv)[n,d,g]

The only per-pair device work is segment-summing the 112-float payload
[gs | q*gs | gv] (32-aligned blocks).  Pairs are sharded by destination
atom (host-side sort), so each of the 8 cores owns N/8 = 1250 atoms and
needs no collectives.  Segment sums are one-hot matmuls on the
TensorEngine: pairs are bucketed into 64-atom windows; the payload tile
[128p x 112] is the stationary operand and a host-precomputed fp8
one-hot [128p x 64n] streams through, accumulating feature-major sums
[112 x 64n] in PSUM.

All TensorEngine-facing tensors are bf16 (fp32 matmul runs ~4x slower
on the PE and disables fast weight load); accumulation stays fp32 in
PSUM and the output is fp32.  The MLP runs on groups of 4 atom tiles
(N=512 matmuls) to amortise per-matmul weight loads.
"""

import sys

if "/opt/trn_rl_repo" not in sys.path:
    sys.path.insert(0, "/opt/trn_rl_repo")

import numpy as np

import concourse.bass as bass
import concourse.bacc as bacc
import concourse.mybir as mybir
import concourse.tile as tile
from concourse.bass_utils import run_bass_kernel_spmd

FP = mybir.dt.float32
BF = mybir.dt.bfloat16
F8 = mybir.dt.float8e4
NP_BF = mybir.dt.np(BF)
NP_F8 = mybir.dt.np(F8)
N_CORES = 8
N_ATOMS = 10000
F = 256
G = 16
H = 64
HID = 256
OUT_F = F + 2  # 258
PW = 112  # payload: gs@0:16 | q*gs@32:48 | gv@64:112 (32-aligned partition starts)
WA = 64  # atoms per one-hot window
APC = N_ATOMS // N_CORES  # 1250 atoms per core
NT = (APC + 127) // 128  # 10 atom tiles per core
NW = 2 * NT  # 20 windows per core
PAD_ATOMS = NT * 128  # 1280
GRP = 2  # atom tiles per MLP group

_ALU = mybir.AluOpType
_ACT = mybir.ActivationFunctionType

_cache = {}


def _build(budgets):
    """Build the SPMD graph. budgets[w] = number of 128-pair tiles for window w."""
    nc = bacc.Bacc(None, target_bir_lowering=False, debug=False)
    t_total = sum(budgets)

    pay_d = nc.dram_tensor("payload", [128, t_total, PW], BF, kind="ExternalInput")
    oh_d = nc.dram_tensor("onehot", [128, t_total, WA], F8, kind="ExternalInput")
    eT_d = nc.dram_tensor("eT", [F, PAD_ATOMS], BF, kind="ExternalInput")
    aghr_d = nc.dram_tensor("aghr", [F, G * H], BF, kind="ExternalInput")
    wgsT_d = nc.dram_tensor("wgsT", [48, F], BF, kind="ExternalInput")
    w1T_d = nc.dram_tensor("w1T", [640, HID], BF, kind="ExternalInput")
    b1_d = nc.dram_tensor("b1", [HID, 1], FP, kind="ExternalInput")
    w2T_d = nc.dram_tensor("w2T", [HID, HID], BF, kind="ExternalInput")
    b2_d = nc.dram_tensor("b2", [HID, 1], FP, kind="ExternalInput")
    w3T_d = nc.dram_tensor("w3T", [HID, OUT_F], BF, kind="ExternalInput")
    b3r_d = nc.dram_tensor("b3r", [1, OUT_F], BF, kind="ExternalInput")
    ident_d = nc.dram_tensor("ident", [128, 128], FP, kind="ExternalInput")
    out_d = nc.dram_tensor("out", [128, 2 * PAD_ATOMS], FP, kind="ExternalOutput")
    out2_d = nc.dram_tensor("out2", [2, PAD_ATOMS], FP, kind="ExternalOutput")

    with tile.TileContext(nc) as tc:
        with (
            tc.tile_pool(name="const", bufs=1) as cpool,
            tc.tile_pool(name="pay", bufs=12) as paypool,
            tc.tile_pool(name="work", bufs=6) as wpool,
            tc.tile_pool(name="grp", bufs=3) as gpool,
            tc.tile_pool(name="ps_seg", bufs=2, space="PSUM") as ps_seg,
            tc.tile_pool(name="ps_big", bufs=2, space="PSUM") as ps_big,
            tc.tile_pool(name="ps_mm", bufs=4, space="PSUM") as ps_mm,
        ):
            eT_sb = cpool.tile([128, 2, PAD_ATOMS], BF)
            nc.gpsimd.dma_start(eT_sb[:], eT_d[:].rearrange("(c p) n -> p c n", p=128))
            aghr_sb = cpool.tile([128, 2, G * H], BF)
            nc.gpsimd.dma_start(aghr_sb[:], aghr_d[:].rearrange("(c p) n -> p c n", p=128))
            wgsT_sb = cpool.tile([48, F], BF)
            nc.gpsimd.dma_start(wgsT_sb[:], wgsT_d[:])
            w1T_sb = cpool.tile([128, 5, HID], BF)
            nc.gpsimd.dma_start(w1T_sb[:], w1T_d[:].rearrange("(c p) n -> p c n", p=128))
            b1_sb = cpool.tile([128, 2, 1], FP)
            nc.gpsimd.dma_start(b1_sb[:], b1_d[:].rearrange("(c p) o -> p c o", p=128))
            w2T_sb = cpool.tile([128, 2, HID], BF)
            nc.gpsimd.dma_start(w2T_sb[:], w2T_d[:].rearrange("(c p) n -> p c n", p=128))
            b2_sb = cpool.tile([128, 2, 1], FP)
            nc.gpsimd.dma_start(b2_sb[:], b2_d[:].rearrange("(c p) o -> p c o", p=128))
            w3T_sb = cpool.tile([128, 2, OUT_F], BF)
            nc.gpsimd.dma_start(w3T_sb[:], w3T_d[:].rearrange("(c p) n -> p c n", p=128))
            b3r_sb = cpool.tile([1, OUT_F], BF)
            nc.gpsimd.dma_start(b3r_sb[:], b3r_d[:])
            ones_sb = cpool.tile([1, 512], BF)
            nc.gpsimd.memset(ones_sb[:], 1.0)
            ident_sb = cpool.tile([128, 128], FP)
            nc.gpsimd.dma_start(ident_sb[:], ident_d[:])
            out_all = cpool.tile([128, 2, PAD_ATOMS], FP)
            o2_all = cpool.tile([2, PAD_ATOMS], FP)

            offs = [sum(budgets[:w]) for w in range(NW)]
            st = [dict() for _ in range(NT)]

            def stage_a(t):
                # segment sums for the two 64-atom windows of atom tile t,
                # gathered into per-tile feature-major collectors
                gq_sb = wpool.tile([48, 128], BF, tag="gq")
                gvT_sb = wpool.tile([48, 128], FP, tag="gvT")
                for half in range(2):
                    w = 2 * t + half
                    B = budgets[w]
                    off = offs[w]
                    pay_sb = paypool.tile([128, B, PW], BF, tag="pay")
                    nc.sync.dma_start(pay_sb[:], pay_d[:, off : off + B, :])
                    oh_sb = paypool.tile([128, B, WA], F8, tag="oh")
                    nc.sync.dma_start(oh_sb[:], oh_d[:, off : off + B, :])
                    sums_ps = ps_seg.tile([PW, WA], FP, tag="seg")
                    for b in range(B):
                        nc.tensor.matmul(
                            sums_ps[:],
                            pay_sb[:, b, :],
                            oh_sb[:, b, :],
                            start=(b == 0),
                            stop=(b == B - 1),
                        )
                    cs = slice(half * WA, (half + 1) * WA)
                    nc.any.tensor_copy(gq_sb[:, cs], sums_ps[0:48, :])
                    nc.any.tensor_copy(gvT_sb[:, cs], sums_ps[64:112, :])
                # atom-major gv sums for the avf broadcast
                gva_ps = ps_mm.tile([128, 48], FP, tag="psmm")
                nc.tensor.transpose(gva_ps[:], gvT_sb[:], ident_sb[0:48, 0:48])
                gvs_bf = wpool.tile([128, 48], BF, tag="gvs")
                nc.any.tensor_copy(gvs_bf[:], gva_ps[:])
                st[t].update(gq=gq_sb, gvs=gvs_bf)

            def stage_b(t):
                # M = E @ AGH for this atom tile, [128, 1024] bf16 (h-major)
                m_sb = wpool.tile([128, G * H], BF, tag="m")
                for nh in range(2):
                    mp = ps_big.tile([128, 512], FP, tag="mps")
                    for kc in range(2):
                        nc.tensor.matmul(
                            mp[:],
                            eT_sb[:, kc, t * 128 : (t + 1) * 128],
                            aghr_sb[:, kc, nh * 512 : (nh + 1) * 512],
                            start=(kc == 0),
                            stop=(kc == 1),
                        )
                    nc.any.tensor_copy(m_sb[:, nh * 512 : (nh + 1) * 512], mp[:])
                st[t]["m"] = m_sb

            def stage_c(t, xT_sb, ti):
                # feature-major MLP input chunks into group tile column block ti:
                # xT 0,1 = radial_emb.T; 2,3 = radial_q.T; 4 = vector_emb.T
                xs = slice(ti * 128, (ti + 1) * 128)
                for c in range(2):
                    mg = ps_mm.tile([128, 128], FP, tag="psmm")
                    nc.tensor.matmul(
                        mg[:], wgsT_sb[0:G, c * 128 : (c + 1) * 128],
                        st[t]["gq"][0:G, :],
                        start=True, stop=True,
                    )
                    nc.vector.tensor_tensor(
                        xT_sb[:, c, xs], mg[:],
                        eT_sb[:, c, t * 128 : (t + 1) * 128], _ALU.mult,
                    )
                for c in range(2):
                    mg = ps_mm.tile([128, 128], FP, tag="psmm")
                    nc.tensor.matmul(
                        mg[:], wgsT_sb[32:48, c * 128 : (c + 1) * 128],
                        st[t]["gq"][32:48, :],
                        start=True, stop=True,
                    )
                    nc.any.tensor_copy(xT_sb[:, 2 + c, xs], mg[:])

            def stage_d1(t, s2p, half):
                # prod[p, d, h, g] = M[p, h, g] * GVs[p, d, g]; avf = sum_g prod
                prod_sb = wpool.tile([128, 3 * G * H], BF, tag="prod")
                m_b = (
                    st[t]["m"][:]
                    .rearrange("p (h g) -> p h g", h=H)
                    .unsqueeze(1)
                    .broadcast_to((128, 3, H, G))
                )
                gv_b = (
                    st[t]["gvs"][:]
                    .rearrange("p (d g) -> p d g", d=3)
                    .unsqueeze(2)
                    .broadcast_to((128, 3, H, G))
                )
                nc.vector.tensor_tensor(
                    prod_sb[:].rearrange("p (d h g) -> p d h g", d=3, h=H),
                    m_b, gv_b, _ALU.mult,
                )
                # bf16 output keeps every reduce operand 2-byte, which is what
                # unlocks the DVE 2x perf mode (one extra bf16 rounding only)
                avf_sb = wpool.tile([128, 3 * H], BF, tag="avf")
                with nc.allow_low_precision("bf16 avf; 2e-2 rel-err budget"):
                    nc.vector.tensor_reduce(
                        avf_sb[:].unsqueeze(2),
                        prod_sb[:].rearrange("p (dh g) -> p dh g", g=G),
                        mybir.AxisListType.X,
                        _ALU.add,
                    )
                sq_sb = wpool.tile([128, 3 * H], FP, tag="sq")
                nc.vector.tensor_tensor(sq_sb[:], avf_sb[:], avf_sb[:], _ALU.mult)
                nc.vector.tensor_reduce(
                    s2p[:, half * H : (half + 1) * H].unsqueeze(2),
                    sq_sb[:].rearrange("p (d h) -> p h d", d=3),
                    mybir.AxisListType.X,
                    _ALU.add,
                )

            def stage_d2(s2p):
                vr2 = wpool.tile([128, 2 * H], FP, tag="vr2")
                nc.scalar.activation(vr2[:], s2p[:], _ACT.Sqrt)
                return vr2

            def stage_d3(vr2, xT_sb, ti0):
                # one transpose serves two tiles' vector_emb
                vT_ps = ps_mm.tile([2 * H, 128], FP, tag="psmm")
                nc.tensor.transpose(vT_ps[:], vr2[:], ident_sb[:])
                for half in range(2):
                    xs = slice((ti0 + half) * 128, (ti0 + half + 1) * 128)
                    nc.vector.tensor_copy(
                        xT_sb[0:H, 4, xs], vT_ps[half * H : (half + 1) * H, :]
                    )

            def stage_e(xT_sb, gn, t0):
                NG = gn * 128
                h1_ps = []
                for m in range(2):
                    ps = ps_mm.tile([128, 512], FP, tag="psmm")
                    for kc in range(5):
                        kk = 128 if kc < 4 else H
                        nc.tensor.matmul(
                            ps[:, 0:NG],
                            w1T_sb[0:kk, kc, m * 128 : (m + 1) * 128],
                            xT_sb[0:kk, kc, 0:NG],
                            start=(kc == 0),
                            stop=(kc == 4),
                        )
                    h1_ps.append(ps)
                h1_sb = wpool.tile([128, 2, 512], BF, tag="h1")
                for m in range(2):
                    nc.scalar.activation(
                        h1_sb[:, m, 0:NG], h1_ps[m][:, 0:NG], _ACT.Gelu,
                        bias=b1_sb[:, m, :],
                    )
                h2_ps = []
                for m in range(2):
                    ps = ps_mm.tile([128, 512], FP, tag="psmm")
                    for kc in range(2):
                        nc.tensor.matmul(
                            ps[:, 0:NG],
                            w2T_sb[:, kc, m * 128 : (m + 1) * 128],
                            h1_sb[:, kc, 0:NG],
                            start=(kc == 0),
                            stop=(kc == 1),
                        )
                    h2_ps.append(ps)
                h2_sb = wpool.tile([128, 2, 512], BF, tag="h2")
                for m in range(2):
                    nc.scalar.activation(
                        h2_sb[:, m, 0:NG], h2_ps[m][:, 0:NG], _ACT.Gelu,
                        bias=b2_sb[:, m, :],
                    )
                os_ = slice(t0 * 128, t0 * 128 + NG)
                for m in range(2):
                    ps = ps_mm.tile([128, 512], FP, tag="psmm")
                    for kc in range(2):
                        nc.tensor.matmul(
                            ps[:, 0:NG],
                            w3T_sb[:, kc, m * 128 : (m + 1) * 128],
                            h2_sb[:, kc, 0:NG],
                            start=(kc == 0),
                            stop=False,
                        )
                    nc.tensor.matmul(
                        ps[:, 0:NG],
                        b3r_sb[0:1, m * 128 : (m + 1) * 128],
                        ones_sb[0:1, 0:NG],
                        start=False,
                        stop=True,
                    )
                    nc.scalar.activation(out_all[:, m, os_], ps[:, 0:NG], _ACT.Copy)
                ps2 = ps_mm.tile([2, 512], FP, tag="psmm")
                for kc in range(2):
                    nc.tensor.matmul(
                        ps2[:, 0:NG],
                        w3T_sb[:, kc, F : F + 2],
                        h2_sb[:, kc, 0:NG],
                        start=(kc == 0),
                        stop=False,
                    )
                nc.tensor.matmul(
                    ps2[:, 0:NG],
                    b3r_sb[0:1, F : F + 2],
                    ones_sb[0:1, 0:NG],
                    start=False,
                    stop=True,
                )
                nc.scalar.activation(o2_all[0:2, os_], ps2[:, 0:NG], _ACT.Copy)

            def front(group):
                gn = len(group)
                t0 = group[0]
                xT_sb = gpool.tile([128, 5, 512], BF, tag="xT")
                s2ps = []
                for t in group:
                    stage_a(t)
                for pi in range(0, gn, 2):
                    pts = group[pi : pi + 2]
                    s2p = wpool.tile([128, 2 * H], FP, tag="s2p")
                    for half, t in enumerate(pts):
                        stage_b(t)
                        stage_c(t, xT_sb, t - t0)
                        stage_d1(t, s2p, half)
                        st[t].clear()
                    s2ps.append((s2p, pi))
                vrs = [(stage_d2(s2p), pi) for s2p, pi in s2ps]
                for vr2, pi in vrs:
                    stage_d3(vr2, xT_sb, pi)
                return xT_sb

            def back(xT_sb, group):
                gn = len(group)
                t0 = group[0]
                stage_e(xT_sb, gn, t0)
                os_ = slice(t0 * 128, t0 * 128 + gn * 128)
                for m in range(2):
                    nc.sync.dma_start(
                        out_d[:, m * PAD_ATOMS + t0 * 128 :
                              m * PAD_ATOMS + t0 * 128 + gn * 128],
                        out_all[:, m, os_],
                    )
                nc.sync.dma_start(out2_d[:, os_], o2_all[:, os_])

            # group-level software pipeline: the MLP of group g is emitted
            # after the front (segment sums / avf) of group g+1, so the PE's
            # MLP work overlaps the vector engine's next-group work
            groups = [list(range(g, min(g + GRP, NT))) for g in range(0, NT, GRP)]
            pending = None
            for group in groups:
                xT_sb = front(group)
                if pending is not None:
                    back(*pending)
                pending = (xT_sb, group)
            back(*pending)

    nc.compile()
    return nc


def _prep(atomic_embedding, partial_charges, pair_indices, gs, gv, agh,
          W_gs, W1, b1, W2, b2, W3, b3):
    E = np.ascontiguousarray(np.asarray(atomic_embedding, dtype=np.float32))
    q = np.asarray(partial_charges, dtype=np.float32).reshape(N_ATOMS)
    idx = np.asarray(pair_indices)[1].astype(np.int64)
    n_pairs = idx.shape[0]
    gs = np.asarray(gs, dtype=np.float32)
    gv = np.asarray(gv, dtype=np.float32).reshape(n_pairs, 3 * G)

    order = np.argsort(idx, kind="stable")
    idx_s = idx[order]
    gs_s = gs[order]
    pay_all = np.zeros((n_pairs, PW), dtype=np.float32)
    pay_all[:, 0:G] = gs_s
    pay_all[:, 32:48] = gs_s * q[idx_s][:, None]
    pay_all[:, 64:112] = gv[order]

    # window boundaries: core k, window w covers atoms [k*APC + w*WA, ...)
    bounds = np.zeros((N_CORES, NW + 1), dtype=np.int64)
    counts = np.zeros((N_CORES, NW), dtype=np.int64)
    for k in range(N_CORES):
        for w in range(NW):
            lo = k * APC + w * WA
            hi = min(k * APC + (w + 1) * WA, (k + 1) * APC)
            bounds[k, w] = np.searchsorted(idx_s, lo)
            if w == NW - 1:
                bounds[k, NW] = np.searchsorted(idx_s, hi)
        counts[k] = np.diff(bounds[k])
    budgets = tuple(
        int(max(1, -(-int(counts[:, w].max()) // 128))) for w in range(NW)
    )
    t_total = sum(budgets)

    # shared params (h-major AGH so the avf product/reduce are contiguous)
    aghr = np.asarray(agh, dtype=np.float32).transpose(0, 2, 1).reshape(F, H * G).astype(NP_BF)
    wgsT16 = np.asarray(W_gs, dtype=np.float32).T.astype(NP_BF)
    wgsT = np.zeros((48, F), dtype=NP_BF)
    wgsT[0:G] = wgsT16
    wgsT[32:48] = wgsT16
    W1 = np.asarray(W1, dtype=np.float32)
    # permute MLP input features: [radial_emb, radial_q, vector_emb], drop vector_q
    W1p = np.concatenate([W1[:, 0:F], W1[:, F + H : 2 * F + H], W1[:, F : F + H]], axis=1)
    w1T = np.zeros((640, HID), dtype=NP_BF)
    w1T[0 : 2 * F + H] = W1p.T.astype(NP_BF)
    w2T = np.asarray(W2, dtype=np.float32).T.astype(NP_BF)
    w3T = np.asarray(W3, dtype=np.float32).T.astype(NP_BF)
    b1v = np.asarray(b1, dtype=np.float32).reshape(HID, 1)
    b2v = np.asarray(b2, dtype=np.float32).reshape(HID, 1)
    b3v = np.asarray(b3, dtype=np.float32).reshape(OUT_F, 1)
    ident = np.eye(128, dtype=np.float32)
    arange_wa = np.arange(WA, dtype=np.int64)

    in_maps = []
    for k in range(N_CORES):
        pay = np.zeros((t_total * 128, PW), dtype=np.float32)
        ohm = np.zeros((t_total * 128, WA), dtype=NP_F8)
        off = 0
        for w in range(NW):
            lo_p, hi_p = bounds[k, w], bounds[k, w + 1]
            cnt = hi_p - lo_p
            r0 = off * 128
            pay[r0 : r0 + cnt] = pay_all[lo_p:hi_p]
            loc = (idx_s[lo_p:hi_p] - (k * APC + w * WA)).astype(np.int64)
            ohm[r0 : r0 + cnt] = (loc[:, None] == arange_wa[None, :]).astype(NP_F8)
            off += budgets[w]
        # partition-major device layouts: [128, t_total, width]
        pay_dev = np.ascontiguousarray(
            pay.reshape(t_total, 128, PW).transpose(1, 0, 2)
        )
        ohm_dev = np.ascontiguousarray(
            ohm.reshape(t_total, 128, WA).transpose(1, 0, 2)
        )
        e_k = np.zeros((PAD_ATOMS, F), dtype=np.float32)
        e_k[0:APC] = E[k * APC : (k + 1) * APC]
        in_maps.append(
            {
                "payload": pay_dev.astype(NP_BF),
                "onehot": ohm_dev,
                "eT": np.ascontiguousarray(e_k.T).astype(NP_BF),
                "aghr": aghr,
                "wgsT": wgsT,
                "w1T": w1T,
                "b1": b1v,
                "w2T": w2T,
                "b2": b2v,
                "w3T": w3T,
                "b3r": b3v.reshape(1, OUT_F).astype(NP_BF),
                "ident": ident,
            }
        )
    return budgets, in_maps


def _run(inputs, trace=False):
    budgets, in_maps = _prep(**inputs)
    if budgets not in _cache:
        _cache[budgets] = _build(list(budgets))
    nc = _cache[budgets]
    res = run_bass_kernel_spmd(
        nc, in_maps, core_ids=list(range(N_CORES)), trace=trace
    )
    outs = []
    for k in range(N_CORES):
        o = res.results[k]["out"].reshape(128, 2, PAD_ATOMS)
        o = o.transpose(1, 0, 2).reshape(2 * 128, PAD_ATOMS)
        outs.append(np.concatenate([o, res.results[k]["out2"]], axis=0))
    full = np.concatenate([o[:, :APC] for o in outs], axis=1).T
    full = np.ascontiguousarray(full, dtype=np.float32)
    delta_q = full[:, 0:1]
    f_out = full[:, 1:2]
    delta_a = full[:, 2:]
    return (delta_a, delta_q, f_out), res


def kernel(**inputs):
    out, _ = _run(inputs, trace=False)
    return out


# revision 24
# speedup vs baseline: 1.3492x; 1.3492x over previous
"""AIMNet2 interaction module on 8 TRN2 NeuronCores.

Strategy: the reference gathers per-pair features with idx_j and
segment-sums with the SAME idx_j.  Within the segment of atom n every
gathered row equals the per-atom value, so the pairwise work collapses:

  radial_emb[n]  = E[n] * (segsum(gs)[n] @ W_gs.T)
  radial_q[n]    = (segsum(q*gs)[n] @ W_gs.T)          (q folded host-side)
  avf_sum[n,h,d] = sum_g (E @ AGH)[n,g,h] * segsum(gv)[n,d,g]

The only per-pair device work is segment-summing the 112-float payload
[gs | q*gs | gv] (32-aligned blocks).  Pairs are sharded by destination
atom (host-side sort), so each of the 8 cores owns N/8 = 1250 atoms and
needs no collectives.  Segment sums are one-hot matmuls on the
TensorEngine: pairs are bucketed into 64-atom windows; the payload tile
[128p x 112] is the stationary operand and a host-precomputed fp8
one-hot [128p x 64n] streams through, accumulating feature-major sums
[112 x 64n] in PSUM.

All TensorEngine-facing tensors are bf16 (fp32 matmul runs ~4x slower
on the PE and disables fast weight load); accumulation stays fp32 in
PSUM and the output is fp32.  The MLP runs on groups of 4 atom tiles
(N=512 matmuls) to amortise per-matmul weight loads.
"""

import sys

if "/opt/trn_rl_repo" not in sys.path:
    sys.path.insert(0, "/opt/trn_rl_repo")

import numpy as np

import concourse.bass as bass
import concourse.bacc as bacc
import concourse.mybir as mybir
import concourse.tile as tile
from concourse.bass_utils import run_bass_kernel_spmd

FP = mybir.dt.float32
BF = mybir.dt.bfloat16
F8 = mybir.dt.float8e4
NP_BF = mybir.dt.np(BF)
NP_F8 = mybir.dt.np(F8)
N_CORES = 8
N_ATOMS = 10000
F = 256
G = 16
H = 64
HID = 256
OUT_F = F + 2  # 258
PW = 112  # payload: gs@0:16 | q*gs@32:48 | gv@64:112 (32-aligned partition starts)
WA = 64  # atoms per one-hot window
APC = N_ATOMS // N_CORES  # 1250 atoms per core
NT = (APC + 127) // 128  # 10 atom tiles per core
NW = 2 * NT  # 20 windows per core
PAD_ATOMS = NT * 128  # 1280
GRP = 4  # atom tiles per MLP group

_ALU = mybir.AluOpType
_ACT = mybir.ActivationFunctionType

_cache = {}


def _build(budgets):
    """Build the SPMD graph. budgets[w] = number of 128-pair tiles for window w."""
    nc = bacc.Bacc(None, target_bir_lowering=False, debug=False)
    t_total = sum(budgets)

    pay_d = nc.dram_tensor("payload", [128, t_total, PW], BF, kind="ExternalInput")
    oh_d = nc.dram_tensor("onehot", [128, t_total, WA], F8, kind="ExternalInput")
    eT_d = nc.dram_tensor("eT", [F, PAD_ATOMS], BF, kind="ExternalInput")
    aghr_d = nc.dram_tensor("aghr", [F, G * H], BF, kind="ExternalInput")
    wgsT_d = nc.dram_tensor("wgsT", [48, F], BF, kind="ExternalInput")
    w1T_d = nc.dram_tensor("w1T", [640, HID], BF, kind="ExternalInput")
    b1_d = nc.dram_tensor("b1", [HID, 1], FP, kind="ExternalInput")
    w2T_d = nc.dram_tensor("w2T", [HID, HID], BF, kind="ExternalInput")
    b2_d = nc.dram_tensor("b2", [HID, 1], FP, kind="ExternalInput")
    w3T_d = nc.dram_tensor("w3T", [HID, OUT_F], BF, kind="ExternalInput")
    b3r_d = nc.dram_tensor("b3r", [1, OUT_F], BF, kind="ExternalInput")
    ident_d = nc.dram_tensor("ident", [128, 128], FP, kind="ExternalInput")
    out_d = nc.dram_tensor("out", [128, 2 * PAD_ATOMS], FP, kind="ExternalOutput")
    out2_d = nc.dram_tensor("out2", [2, PAD_ATOMS], FP, kind="ExternalOutput")

    with tile.TileContext(nc) as tc:
        with (
            tc.tile_pool(name="const", bufs=1) as cpool,
            tc.tile_pool(name="pay", bufs=12) as paypool,
            tc.tile_pool(name="work", bufs=6) as wpool,
            tc.tile_pool(name="grp", bufs=3) as gpool,
            tc.tile_pool(name="ps_seg", bufs=2, space="PSUM") as ps_seg,
            tc.tile_pool(name="ps_big", bufs=2, space="PSUM") as ps_big,
            tc.tile_pool(name="ps_mm", bufs=4, space="PSUM") as ps_mm,
        ):
            eT_sb = cpool.tile([128, 2, PAD_ATOMS], BF)
            nc.gpsimd.dma_start(eT_sb[:], eT_d[:].rearrange("(c p) n -> p c n", p=128))
            aghr_sb = cpool.tile([128, 2, G * H], BF)
            nc.gpsimd.dma_start(aghr_sb[:], aghr_d[:].rearrange("(c p) n -> p c n", p=128))
            wgsT_sb = cpool.tile([48, F], BF)
            nc.gpsimd.dma_start(wgsT_sb[:], wgsT_d[:])
            w1T_sb = cpool.tile([128, 5, HID], BF)
            nc.gpsimd.dma_start(w1T_sb[:], w1T_d[:].rearrange("(c p) n -> p c n", p=128))
            b1_sb = cpool.tile([128, 2, 1], FP)
            nc.gpsimd.dma_start(b1_sb[:], b1_d[:].rearrange("(c p) o -> p c o", p=128))
            w2T_sb = cpool.tile([128, 2, HID], BF)
            nc.gpsimd.dma_start(w2T_sb[:], w2T_d[:].rearrange("(c p) n -> p c n", p=128))
            b2_sb = cpool.tile([128, 2, 1], FP)
            nc.gpsimd.dma_start(b2_sb[:], b2_d[:].rearrange("(c p) o -> p c o", p=128))
            w3T_sb = cpool.tile([128, 2, OUT_F], BF)
            nc.gpsimd.dma_start(w3T_sb[:], w3T_d[:].rearrange("(c p) n -> p c n", p=128))
            b3r_sb = cpool.tile([1, OUT_F], BF)
            nc.gpsimd.dma_start(b3r_sb[:], b3r_d[:])
            ones_sb = cpool.tile([1, 512], BF)
            nc.gpsimd.memset(ones_sb[:], 1.0)
            ident_sb = cpool.tile([128, 128], FP)
            nc.gpsimd.dma_start(ident_sb[:], ident_d[:])
            out_all = cpool.tile([128, 2, PAD_ATOMS], FP)
            o2_all = cpool.tile([2, PAD_ATOMS], FP)

            offs = [sum(budgets[:w]) for w in range(NW)]
            st = [dict() for _ in range(NT)]

            def stage_a(t):
                # segment sums for the two 64-atom windows of atom tile t,
                # gathered into per-tile feature-major collectors
                gq_sb = wpool.tile([48, 128], BF, tag="gq")
                gvT_sb = wpool.tile([48, 128], FP, tag="gvT")
                for half in range(2):
                    w = 2 * t + half
                    B = budgets[w]
                    off = offs[w]
                    pay_sb = paypool.tile([128, B, PW], BF, tag="pay")
                    nc.sync.dma_start(pay_sb[:], pay_d[:, off : off + B, :])
                    oh_sb = paypool.tile([128, B, WA], F8, tag="oh")
                    nc.sync.dma_start(oh_sb[:], oh_d[:, off : off + B, :])
                    sums_ps = ps_seg.tile([PW, WA], FP, tag="seg")
                    for b in range(B):
                        nc.tensor.matmul(
                            sums_ps[:],
                            pay_sb[:, b, :],
                            oh_sb[:, b, :],
                            start=(b == 0),
                            stop=(b == B - 1),
                        )
                    cs = slice(half * WA, (half + 1) * WA)
                    nc.any.tensor_copy(gq_sb[:, cs], sums_ps[0:48, :])
                    nc.any.tensor_copy(gvT_sb[:, cs], sums_ps[64:112, :])
                # atom-major gv sums for the avf broadcast
                gva_ps = ps_mm.tile([128, 48], FP, tag="psmm")
                nc.tensor.transpose(gva_ps[:], gvT_sb[:], ident_sb[0:48, 0:48])
                gvs_bf = wpool.tile([128, 48], BF, tag="gvs")
                nc.any.tensor_copy(gvs_bf[:], gva_ps[:])
                st[t].update(gq=gq_sb, gvs=gvs_bf)

            def stage_b(t):
                # M = E @ AGH for this atom tile, [128, 1024] bf16 (h-major)
                m_sb = wpool.tile([128, G * H], BF, tag="m")
                for nh in range(2):
                    mp = ps_big.tile([128, 512], FP, tag="mps")
                    for kc in range(2):
                        nc.tensor.matmul(
                            mp[:],
                            eT_sb[:, kc, t * 128 : (t + 1) * 128],
                            aghr_sb[:, kc, nh * 512 : (nh + 1) * 512],
                            start=(kc == 0),
                            stop=(kc == 1),
                        )
                    nc.any.tensor_copy(m_sb[:, nh * 512 : (nh + 1) * 512], mp[:])
                st[t]["m"] = m_sb

            def stage_c(t, xT_sb, ti):
                # feature-major MLP input chunks into group tile column block ti:
                # xT 0,1 = radial_emb.T; 2,3 = radial_q.T; 4 = vector_emb.T
                xs = slice(ti * 128, (ti + 1) * 128)
                for c in range(2):
                    mg = ps_mm.tile([128, 128], FP, tag="psmm")
                    nc.tensor.matmul(
                        mg[:], wgsT_sb[0:G, c * 128 : (c + 1) * 128],
                        st[t]["gq"][0:G, :],
                        start=True, stop=True,
                    )
                    nc.vector.tensor_tensor(
                        xT_sb[:, c, xs], mg[:],
                        eT_sb[:, c, t * 128 : (t + 1) * 128], _ALU.mult,
                    )
                for c in range(2):
                    mg = ps_mm.tile([128, 128], FP, tag="psmm")
                    nc.tensor.matmul(
                        mg[:], wgsT_sb[32:48, c * 128 : (c + 1) * 128],
                        st[t]["gq"][32:48, :],
                        start=True, stop=True,
                    )
                    nc.any.tensor_copy(xT_sb[:, 2 + c, xs], mg[:])

            def stage_d1(t, s2p, half):
                # prod[p, d, h, g] = M[p, h, g] * GVs[p, d, g]; avf = sum_g prod
                prod_sb = wpool.tile([128, 3 * G * H], BF, tag="prod")
                m_b = (
                    st[t]["m"][:]
                    .rearrange("p (h g) -> p h g", h=H)
                    .unsqueeze(1)
                    .broadcast_to((128, 3, H, G))
                )
                gv_b = (
                    st[t]["gvs"][:]
                    .rearrange("p (d g) -> p d g", d=3)
                    .unsqueeze(2)
                    .broadcast_to((128, 3, H, G))
                )
                nc.vector.tensor_tensor(
                    prod_sb[:].rearrange("p (d h g) -> p d h g", d=3, h=H),
                    m_b, gv_b, _ALU.mult,
                )
                # bf16 output keeps every reduce operand 2-byte, which is what
                # unlocks the DVE 2x perf mode (one extra bf16 rounding only)
                avf_sb = wpool.tile([128, 3 * H], BF, tag="avf")
                with nc.allow_low_precision("bf16 avf; 2e-2 rel-err budget"):
                    nc.vector.tensor_reduce(
                        avf_sb[:].unsqueeze(2),
                        prod_sb[:].rearrange("p (dh g) -> p dh g", g=G),
                        mybir.AxisListType.X,
                        _ALU.add,
                    )
                sq_sb = wpool.tile([128, 3 * H], FP, tag="sq")
                nc.vector.tensor_tensor(sq_sb[:], avf_sb[:], avf_sb[:], _ALU.mult)
                nc.vector.tensor_reduce(
                    s2p[:, half * H : (half + 1) * H].unsqueeze(2),
                    sq_sb[:].rearrange("p (d h) -> p h d", d=3),
                    mybir.AxisListType.X,
                    _ALU.add,
                )

            def stage_d2(s2p):
                vr2 = wpool.tile([128, 2 * H], FP, tag="vr2")
                nc.scalar.activation(vr2[:], s2p[:], _ACT.Sqrt)
                return vr2

            def stage_d3(vr2, xT_sb, ti0):
                # one transpose serves two tiles' vector_emb
                vT_ps = ps_mm.tile([2 * H, 128], FP, tag="psmm")
                nc.tensor.transpose(vT_ps[:], vr2[:], ident_sb[:])
                for half in range(2):
                    xs = slice((ti0 + half) * 128, (ti0 + half + 1) * 128)
                    nc.vector.tensor_copy(
                        xT_sb[0:H, 4, xs], vT_ps[half * H : (half + 1) * H, :]
                    )

            def stage_e(xT_sb, gn, t0):
                NG = gn * 128
                h1_ps = []
                for m in range(2):
                    ps = ps_mm.tile([128, 512], FP, tag="psmm")
                    for kc in range(5):
                        kk = 128 if kc < 4 else H
                        nc.tensor.matmul(
                            ps[:, 0:NG],
                            w1T_sb[0:kk, kc, m * 128 : (m + 1) * 128],
                            xT_sb[0:kk, kc, 0:NG],
                            start=(kc == 0),
                            stop=(kc == 4),
                        )
                    h1_ps.append(ps)
                h1_sb = wpool.tile([128, 2, 512], BF, tag="h1")
                for m in range(2):
                    nc.scalar.activation(
                        h1_sb[:, m, 0:NG], h1_ps[m][:, 0:NG], _ACT.Gelu,
                        bias=b1_sb[:, m, :],
                    )
                h2_ps = []
                for m in range(2):
                    ps = ps_mm.tile([128, 512], FP, tag="psmm")
                    for kc in range(2):
                        nc.tensor.matmul(
                            ps[:, 0:NG],
                            w2T_sb[:, kc, m * 128 : (m + 1) * 128],
                            h1_sb[:, kc, 0:NG],
                            start=(kc == 0),
                            stop=(kc == 1),
                        )
                    h2_ps.append(ps)
                h2_sb = wpool.tile([128, 2, 512], BF, tag="h2")
                for m in range(2):
                    nc.scalar.activation(
                        h2_sb[:, m, 0:NG], h2_ps[m][:, 0:NG], _ACT.Gelu,
                        bias=b2_sb[:, m, :],
                    )
                os_ = slice(t0 * 128, t0 * 128 + NG)
                for m in range(2):
                    ps = ps_mm.tile([128, 512], FP, tag="psmm")
                    for kc in range(2):
                        nc.tensor.matmul(
                            ps[:, 0:NG],
                            w3T_sb[:, kc, m * 128 : (m + 1) * 128],
                            h2_sb[:, kc, 0:NG],
                            start=(kc == 0),
                            stop=False,
                        )
                    nc.tensor.matmul(
                        ps[:, 0:NG],
                        b3r_sb[0:1, m * 128 : (m + 1) * 128],
                        ones_sb[0:1, 0:NG],
                        start=False,
                        stop=True,
                    )
                    nc.scalar.activation(out_all[:, m, os_], ps[:, 0:NG], _ACT.Copy)
                ps2 = ps_mm.tile([2, 512], FP, tag="psmm")
                for kc in range(2):
                    nc.tensor.matmul(
                        ps2[:, 0:NG],
                        w3T_sb[:, kc, F : F + 2],
                        h2_sb[:, kc, 0:NG],
                        start=(kc == 0),
                        stop=False,
                    )
                nc.tensor.matmul(
                    ps2[:, 0:NG],
                    b3r_sb[0:1, F : F + 2],
                    ones_sb[0:1, 0:NG],
                    start=False,
                    stop=True,
                )
                nc.scalar.activation(o2_all[0:2, os_], ps2[:, 0:NG], _ACT.Copy)

            def front(group):
                gn = len(group)
                t0 = group[0]
                xT_sb = gpool.tile([128, 5, 512], BF, tag="xT")
                s2ps = []
                for t in group:
                    stage_a(t)
                for pi in range(0, gn, 2):
                    pts = group[pi : pi + 2]
                    s2p = wpool.tile([128, 2 * H], FP, tag="s2p")
                    for half, t in enumerate(pts):
                        stage_b(t)
                        stage_c(t, xT_sb, t - t0)
                        stage_d1(t, s2p, half)
                        st[t].clear()
                    s2ps.append((s2p, pi))
                vrs = [(stage_d2(s2p), pi) for s2p, pi in s2ps]
                for vr2, pi in vrs:
                    stage_d3(vr2, xT_sb, pi)
                return xT_sb

            def back(xT_sb, group):
                gn = len(group)
                t0 = group[0]
                stage_e(xT_sb, gn, t0)
                os_ = slice(t0 * 128, t0 * 128 + gn * 128)
                for m in range(2):
                    nc.sync.dma_start(
                        out_d[:, m * PAD_ATOMS + t0 * 128 :
                              m * PAD_ATOMS + t0 * 128 + gn * 128],
                        out_all[:, m, os_],
                    )
                nc.sync.dma_start(out2_d[:, os_], o2_all[:, os_])

            # group-level software pipeline: the MLP of group g is emitted
            # after the front (segment sums / avf) of group g+1, so the PE's
            # MLP work overlaps the vector engine's next-group work
            groups = [list(range(g, min(g + GRP, NT))) for g in range(0, NT, GRP)]
            pending = None
            for group in groups:
                xT_sb = front(group)
                if pending is not None:
                    back(*pending)
                pending = (xT_sb, group)
            back(*pending)

    nc.compile()
    return nc


def _prep(atomic_embedding, partial_charges, pair_indices, gs, gv, agh,
          W_gs, W1, b1, W2, b2, W3, b3):
    E = np.ascontiguousarray(np.asarray(atomic_embedding, dtype=np.float32))
    q = np.asarray(partial_charges, dtype=np.float32).reshape(N_ATOMS)
    idx = np.asarray(pair_indices)[1].astype(np.int64)
    n_pairs = idx.shape[0]
    gs = np.asarray(gs, dtype=np.float32)
    gv = np.asarray(gv, dtype=np.float32).reshape(n_pairs, 3 * G)

    order = np.argsort(idx, kind="stable")
    idx_s = idx[order]
    gs_s = gs[order]
    pay_all = np.zeros((n_pairs, PW), dtype=np.float32)
    pay_all[:, 0:G] = gs_s
    pay_all[:, 32:48] = gs_s * q[idx_s][:, None]
    pay_all[:, 64:112] = gv[order]

    # window boundaries: core k, window w covers atoms [k*APC + w*WA, ...)
    bounds = np.zeros((N_CORES, NW + 1), dtype=np.int64)
    counts = np.zeros((N_CORES, NW), dtype=np.int64)
    for k in range(N_CORES):
        for w in range(NW):
            lo = k * APC + w * WA
            hi = min(k * APC + (w + 1) * WA, (k + 1) * APC)
            bounds[k, w] = np.searchsorted(idx_s, lo)
            if w == NW - 1:
                bounds[k, NW] = np.searchsorted(idx_s, hi)
        counts[k] = np.diff(bounds[k])
    budgets = tuple(
        int(max(1, -(-int(counts[:, w].max()) // 128))) for w in range(NW)
    )
    t_total = sum(budgets)

    # shared params (h-major AGH so the avf product/reduce are contiguous)
    aghr = np.asarray(agh, dtype=np.float32).transpose(0, 2, 1).reshape(F, H * G).astype(NP_BF)
    wgsT16 = np.asarray(W_gs, dtype=np.float32).T.astype(NP_BF)
    wgsT = np.zeros((48, F), dtype=NP_BF)
    wgsT[0:G] = wgsT16
    wgsT[32:48] = wgsT16
    W1 = np.asarray(W1, dtype=np.float32)
    # permute MLP input features: [radial_emb, radial_q, vector_emb], drop vector_q
    W1p = np.concatenate([W1[:, 0:F], W1[:, F + H : 2 * F + H], W1[:, F : F + H]], axis=1)
    w1T = np.zeros((640, HID), dtype=NP_BF)
    w1T[0 : 2 * F + H] = W1p.T.astype(NP_BF)
    w2T = np.asarray(W2, dtype=np.float32).T.astype(NP_BF)
    w3T = np.asarray(W3, dtype=np.float32).T.astype(NP_BF)
    b1v = np.asarray(b1, dtype=np.float32).reshape(HID, 1)
    b2v = np.asarray(b2, dtype=np.float32).reshape(HID, 1)
    b3v = np.asarray(b3, dtype=np.float32).reshape(OUT_F, 1)
    ident = np.eye(128, dtype=np.float32)
    arange_wa = np.arange(WA, dtype=np.int64)

    in_maps = []
    for k in range(N_CORES):
        pay = np.zeros((t_total * 128, PW), dtype=np.float32)
        ohm = np.zeros((t_total * 128, WA), dtype=NP_F8)
        off = 0
        for w in range(NW):
            lo_p, hi_p = bounds[k, w], bounds[k, w + 1]
            cnt = hi_p - lo_p
            r0 = off * 128
            pay[r0 : r0 + cnt] = pay_all[lo_p:hi_p]
            loc = (idx_s[lo_p:hi_p] - (k * APC + w * WA)).astype(np.int64)
            ohm[r0 : r0 + cnt] = (loc[:, None] == arange_wa[None, :]).astype(NP_F8)
            off += budgets[w]
        # partition-major device layouts: [128, t_total, width]
        pay_dev = np.ascontiguousarray(
            pay.reshape(t_total, 128, PW).transpose(1, 0, 2)
        )
        ohm_dev = np.ascontiguousarray(
            ohm.reshape(t_total, 128, WA).transpose(1, 0, 2)
        )
        e_k = np.zeros((PAD_ATOMS, F), dtype=np.float32)
        e_k[0:APC] = E[k * APC : (k + 1) * APC]
        in_maps.append(
            {
                "payload": pay_dev.astype(NP_BF),
                "onehot": ohm_dev,
                "eT": np.ascontiguousarray(e_k.T).astype(NP_BF),
                "aghr": aghr,
                "wgsT": wgsT,
                "w1T": w1T,
                "b1": b1v,
                "w2T": w2T,
                "b2": b2v,
                "w3T": w3T,
                "b3r": b3v.reshape(1, OUT_F).astype(NP_BF),
                "ident": ident,
            }
        )
    return budgets, in_maps


def _run(inputs, trace=False):
    budgets, in_maps = _prep(**inputs)
    if budgets not in _cache:
        _cache[budgets] = _build(list(budgets))
    nc = _cache[budgets]
    res = run_bass_kernel_spmd(
        nc, in_maps, core_ids=list(range(N_CORES)), trace=trace
    )
    outs = []
    for k in range(N_CORES):
        o = res.results[k]["out"].reshape(128, 2, PAD_ATOMS)
        o = o.transpose(1, 0, 2).reshape(2 * 128, PAD_ATOMS)
        outs.append(np.concatenate([o, res.results[k]["out2"]], axis=0))
    full = np.concatenate([o[:, :APC] for o in outs], axis=1).T
    full = np.ascontiguousarray(full, dtype=np.float32)
    delta_q = full[:, 0:1]
    f_out = full[:, 1:2]
    delta_a = full[:, 2:]
    return (delta_a, delta_q, f_out), res


def kernel(**inputs):
    out, _ = _run(inputs, trace=False)
    return out


# revision 25
# speedup vs baseline: 1.4034x; 1.0402x over previous
"""AIMNet2 interaction module on 8 TRN2 NeuronCores.

Strategy: the reference gathers per-pair features with idx_j and
segment-sums with the SAME idx_j.  Within the segment of atom n every
gathered row equals the per-atom value, so the pairwise work collapses:

  radial_emb[n]  = E[n] * (segsum(gs)[n] @ W_gs.T)
  radial_q[n]    = (segsum(q*gs)[n] @ W_gs.T)          (q folded host-side)
  avf_sum[n,h,d] = sum_g (E @ AGH)[n,g,h] * segsum(gv)[n,d,g]

The only per-pair device work is segment-summing the 112-float payload
[gs | q*gs | gv] (32-aligned blocks).  Pairs are sharded by destination
atom (host-side sort), so each of the 8 cores owns N/8 = 1250 atoms and
needs no collectives.  Segment sums are one-hot matmuls on the
TensorEngine: pairs are bucketed into 64-atom windows; the payload tile
[128p x 112] is the stationary operand and a host-precomputed fp8
one-hot [128p x 64n] streams through, accumulating feature-major sums
[112 x 64n] in PSUM.

All TensorEngine-facing tensors are bf16 (fp32 matmul runs ~4x slower
on the PE and disables fast weight load); accumulation stays fp32 in
PSUM and the output is fp32.  The MLP runs on groups of 4 atom tiles
(N=512 matmuls) to amortise per-matmul weight loads.
"""

import sys

if "/opt/trn_rl_repo" not in sys.path:
    sys.path.insert(0, "/opt/trn_rl_repo")

import numpy as np

import concourse.bass as bass
import concourse.bacc as bacc
import concourse.mybir as mybir
import concourse.tile as tile
from concourse.bass_utils import run_bass_kernel_spmd

FP = mybir.dt.float32
BF = mybir.dt.bfloat16
F8 = mybir.dt.float8e4
NP_BF = mybir.dt.np(BF)
NP_F8 = mybir.dt.np(F8)
N_CORES = 8
N_ATOMS = 10000
F = 256
G = 16
H = 64
HID = 256
OUT_F = F + 2  # 258
PW = 112  # payload: gs@0:16 | q*gs@32:48 | gv@64:112 (32-aligned partition starts)
WA = 64  # atoms per one-hot window
APC = N_ATOMS // N_CORES  # 1250 atoms per core
NT = (APC + 127) // 128  # 10 atom tiles per core
NW = 2 * NT  # 20 windows per core
PAD_ATOMS = NT * 128  # 1280
GRP = 4  # atom tiles per MLP group

_ALU = mybir.AluOpType
_ACT = mybir.ActivationFunctionType

_cache = {}


def _build(budgets):
    """Build the SPMD graph. budgets[w] = number of 128-pair tiles for window w."""
    nc = bacc.Bacc(None, target_bir_lowering=False, debug=False)
    t_total = sum(budgets)

    pay_d = nc.dram_tensor("payload", [128, t_total, PW], BF, kind="ExternalInput")
    oh_d = nc.dram_tensor("onehot", [128, t_total, WA], F8, kind="ExternalInput")
    eT_d = nc.dram_tensor("eT", [F, PAD_ATOMS], BF, kind="ExternalInput")
    aghr_d = nc.dram_tensor("aghr", [F, G * H], BF, kind="ExternalInput")
    wgsT_d = nc.dram_tensor("wgsT", [48, F], BF, kind="ExternalInput")
    w1T_d = nc.dram_tensor("w1T", [640, HID], BF, kind="ExternalInput")
    b1_d = nc.dram_tensor("b1", [HID, 1], FP, kind="ExternalInput")
    w2T_d = nc.dram_tensor("w2T", [HID, HID], BF, kind="ExternalInput")
    b2_d = nc.dram_tensor("b2", [HID, 1], FP, kind="ExternalInput")
    w3T_d = nc.dram_tensor("w3T", [HID, OUT_F], BF, kind="ExternalInput")
    b3r_d = nc.dram_tensor("b3r", [1, OUT_F], BF, kind="ExternalInput")
    ident_d = nc.dram_tensor("ident", [128, 128], FP, kind="ExternalInput")
    out_d = nc.dram_tensor("out", [128, 2 * PAD_ATOMS], FP, kind="ExternalOutput")
    out2_d = nc.dram_tensor("out2", [2, PAD_ATOMS], FP, kind="ExternalOutput")

    with tile.TileContext(nc) as tc:
        with (
            tc.tile_pool(name="const", bufs=1) as cpool,
            tc.tile_pool(name="pay", bufs=12) as paypool,
            tc.tile_pool(name="work", bufs=6) as wpool,
            tc.tile_pool(name="grp", bufs=3) as gpool,
            tc.tile_pool(name="ps_seg", bufs=2, space="PSUM") as ps_seg,
            tc.tile_pool(name="ps_big", bufs=2, space="PSUM") as ps_big,
            tc.tile_pool(name="ps_mm", bufs=4, space="PSUM") as ps_mm,
        ):
            eT_sb = cpool.tile([128, 2, PAD_ATOMS], BF)
            nc.gpsimd.dma_start(eT_sb[:], eT_d[:].rearrange("(c p) n -> p c n", p=128))
            aghr_sb = cpool.tile([128, 2, G * H], BF)
            nc.gpsimd.dma_start(aghr_sb[:], aghr_d[:].rearrange("(c p) n -> p c n", p=128))
            wgsT_sb = cpool.tile([48, F], BF)
            nc.gpsimd.dma_start(wgsT_sb[:], wgsT_d[:])
            w1T_sb = cpool.tile([128, 5, HID], BF)
            nc.gpsimd.dma_start(w1T_sb[:], w1T_d[:].rearrange("(c p) n -> p c n", p=128))
            b1_sb = cpool.tile([128, 2, 1], FP)
            nc.gpsimd.dma_start(b1_sb[:], b1_d[:].rearrange("(c p) o -> p c o", p=128))
            w2T_sb = cpool.tile([128, 2, HID], BF)
            nc.gpsimd.dma_start(w2T_sb[:], w2T_d[:].rearrange("(c p) n -> p c n", p=128))
            b2_sb = cpool.tile([128, 2, 1], FP)
            nc.gpsimd.dma_start(b2_sb[:], b2_d[:].rearrange("(c p) o -> p c o", p=128))
            w3T_sb = cpool.tile([128, 2, OUT_F], BF)
            nc.gpsimd.dma_start(w3T_sb[:], w3T_d[:].rearrange("(c p) n -> p c n", p=128))
            b3r_sb = cpool.tile([1, OUT_F], BF)
            nc.gpsimd.dma_start(b3r_sb[:], b3r_d[:])
            ones_sb = cpool.tile([1, 512], BF)
            nc.gpsimd.memset(ones_sb[:], 1.0)
            ident_sb = cpool.tile([128, 128], FP)
            nc.gpsimd.dma_start(ident_sb[:], ident_d[:])
            out_all = cpool.tile([128, 2, PAD_ATOMS], FP)
            o2_all = cpool.tile([2, PAD_ATOMS], FP)

            offs = [sum(budgets[:w]) for w in range(NW)]
            st = [dict() for _ in range(NT)]

            def stage_a(t):
                # segment sums for the two 64-atom windows of atom tile t,
                # gathered into per-tile feature-major collectors
                gqa_sb = wpool.tile([128, 48], FP, tag="gqa")
                gvs_bf = wpool.tile([128, 48], BF, tag="gvs")
                for half in range(2):
                    w = 2 * t + half
                    B = budgets[w]
                    off = offs[w]
                    pay_sb = paypool.tile([128, B, PW], BF, tag="pay")
                    nc.sync.dma_start(pay_sb[:], pay_d[:, off : off + B, :])
                    oh_sb = paypool.tile([128, B, WA], F8, tag="oh")
                    nc.sync.dma_start(oh_sb[:], oh_d[:, off : off + B, :])
                    # one-hot stationary (64-col weight load), payload moving:
                    # sums come out atom-major [64, 112]
                    sums_ps = ps_seg.tile([WA, PW], FP, tag="seg")
                    for b in range(B):
                        nc.tensor.matmul(
                            sums_ps[:],
                            oh_sb[:, b, :],
                            pay_sb[:, b, :],
                            start=(b == 0),
                            stop=(b == B - 1),
                        )
                    rs = slice(half * WA, (half + 1) * WA)
                    nc.any.tensor_copy(gqa_sb[rs, :], sums_ps[:, 0:48])
                    nc.any.tensor_copy(gvs_bf[rs, :], sums_ps[:, 64:112])
                # feature-major gs / q*gs sums for the radial matmuls
                gqT_ps = ps_mm.tile([48, 128], FP, tag="psmm")
                nc.tensor.transpose(gqT_ps[:], gqa_sb[:], ident_sb[:])
                gq_sb = wpool.tile([48, 128], BF, tag="gq")
                nc.any.tensor_copy(gq_sb[:], gqT_ps[:])
                st[t].update(gq=gq_sb, gvs=gvs_bf)

            def stage_b(t):
                # M = E @ AGH for this atom tile, [128, 1024] bf16 (h-major)
                m_sb = wpool.tile([128, G * H], BF, tag="m")
                for nh in range(2):
                    mp = ps_big.tile([128, 512], FP, tag="mps")
                    for kc in range(2):
                        nc.tensor.matmul(
                            mp[:],
                            eT_sb[:, kc, t * 128 : (t + 1) * 128],
                            aghr_sb[:, kc, nh * 512 : (nh + 1) * 512],
                            start=(kc == 0),
                            stop=(kc == 1),
                        )
                    nc.any.tensor_copy(m_sb[:, nh * 512 : (nh + 1) * 512], mp[:])
                st[t]["m"] = m_sb

            def stage_c(t, xT_sb, ti):
                # feature-major MLP input chunks into group tile column block ti:
                # xT 0,1 = radial_emb.T; 2,3 = radial_q.T; 4 = vector_emb.T
                xs = slice(ti * 128, (ti + 1) * 128)
                for c in range(2):
                    mg = ps_mm.tile([128, 128], FP, tag="psmm")
                    nc.tensor.matmul(
                        mg[:], wgsT_sb[0:G, c * 128 : (c + 1) * 128],
                        st[t]["gq"][0:G, :],
                        start=True, stop=True,
                    )
                    nc.vector.tensor_tensor(
                        xT_sb[:, c, xs], mg[:],
                        eT_sb[:, c, t * 128 : (t + 1) * 128], _ALU.mult,
                    )
                for c in range(2):
                    mg = ps_mm.tile([128, 128], FP, tag="psmm")
                    nc.tensor.matmul(
                        mg[:], wgsT_sb[32:48, c * 128 : (c + 1) * 128],
                        st[t]["gq"][32:48, :],
                        start=True, stop=True,
                    )
                    nc.any.tensor_copy(xT_sb[:, 2 + c, xs], mg[:])

            def stage_d1(t, s2p, half):
                # prod[p, d, h, g] = M[p, h, g] * GVs[p, d, g]; avf = sum_g prod
                prod_sb = wpool.tile([128, 3 * G * H], BF, tag="prod")
                m_b = (
                    st[t]["m"][:]
                    .rearrange("p (h g) -> p h g", h=H)
                    .unsqueeze(1)
                    .broadcast_to((128, 3, H, G))
                )
                gv_b = (
                    st[t]["gvs"][:]
                    .rearrange("p (d g) -> p d g", d=3)
                    .unsqueeze(2)
                    .broadcast_to((128, 3, H, G))
                )
                nc.vector.tensor_tensor(
                    prod_sb[:].rearrange("p (d h g) -> p d h g", d=3, h=H),
                    m_b, gv_b, _ALU.mult,
                )
                # bf16 output keeps every reduce operand 2-byte, which is what
                # unlocks the DVE 2x perf mode (one extra bf16 rounding only)
                avf_sb = wpool.tile([128, 3 * H], BF, tag="avf")
                with nc.allow_low_precision("bf16 avf; 2e-2 rel-err budget"):
                    nc.vector.tensor_reduce(
                        avf_sb[:].unsqueeze(2),
                        prod_sb[:].rearrange("p (dh g) -> p dh g", g=G),
                        mybir.AxisListType.X,
                        _ALU.add,
                    )
                sq_sb = wpool.tile([128, 3 * H], FP, tag="sq")
                nc.vector.tensor_tensor(sq_sb[:], avf_sb[:], avf_sb[:], _ALU.mult)
                nc.vector.tensor_reduce(
                    s2p[:, half * H : (half + 1) * H].unsqueeze(2),
                    sq_sb[:].rearrange("p (d h) -> p h d", d=3),
                    mybir.AxisListType.X,
                    _ALU.add,
                )

            def stage_d2(s2p):
                vr2 = wpool.tile([128, 2 * H], FP, tag="vr2")
                nc.scalar.activation(vr2[:], s2p[:], _ACT.Sqrt)
                return vr2

            def stage_d3(vr2, xT_sb, ti0):
                # one transpose serves two tiles' vector_emb
                vT_ps = ps_mm.tile([2 * H, 128], FP, tag="psmm")
                nc.tensor.transpose(vT_ps[:], vr2[:], ident_sb[:])
                for half in range(2):
                    xs = slice((ti0 + half) * 128, (ti0 + half + 1) * 128)
                    nc.vector.tensor_copy(
                        xT_sb[0:H, 4, xs], vT_ps[half * H : (half + 1) * H, :]
                    )

            def stage_e(xT_sb, gn, t0):
                NG = gn * 128
                h1_ps = []
                for m in range(2):
                    ps = ps_mm.tile([128, 512], FP, tag="psmm")
                    for kc in range(5):
                        kk = 128 if kc < 4 else H
                        nc.tensor.matmul(
                            ps[:, 0:NG],
                            w1T_sb[0:kk, kc, m * 128 : (m + 1) * 128],
                            xT_sb[0:kk, kc, 0:NG],
                            start=(kc == 0),
                            stop=(kc == 4),
                        )
                    h1_ps.append(ps)
                h1_sb = wpool.tile([128, 2, 512], BF, tag="h1")
                for m in range(2):
                    nc.scalar.activation(
                        h1_sb[:, m, 0:NG], h1_ps[m][:, 0:NG], _ACT.Gelu,
                        bias=b1_sb[:, m, :],
                    )
                h2_ps = []
                for m in range(2):
                    ps = ps_mm.tile([128, 512], FP, tag="psmm")
                    for kc in range(2):
                        nc.tensor.matmul(
                            ps[:, 0:NG],
                            w2T_sb[:, kc, m * 128 : (m + 1) * 128],
                            h1_sb[:, kc, 0:NG],
                            start=(kc == 0),
                            stop=(kc == 1),
                        )
                    h2_ps.append(ps)
                h2_sb = wpool.tile([128, 2, 512], BF, tag="h2")
                for m in range(2):
                    nc.scalar.activation(
                        h2_sb[:, m, 0:NG], h2_ps[m][:, 0:NG], _ACT.Gelu,
                        bias=b2_sb[:, m, :],
                    )
                os_ = slice(t0 * 128, t0 * 128 + NG)
                for m in range(2):
                    ps = ps_mm.tile([128, 512], FP, tag="psmm")
                    for kc in range(2):
                        nc.tensor.matmul(
                            ps[:, 0:NG],
                            w3T_sb[:, kc, m * 128 : (m + 1) * 128],
                            h2_sb[:, kc, 0:NG],
                            start=(kc == 0),
                            stop=False,
                        )
                    nc.tensor.matmul(
                        ps[:, 0:NG],
                        b3r_sb[0:1, m * 128 : (m + 1) * 128],
                        ones_sb[0:1, 0:NG],
                        start=False,
                        stop=True,
                    )
                    nc.scalar.activation(out_all[:, m, os_], ps[:, 0:NG], _ACT.Copy)
                ps2 = ps_mm.tile([2, 512], FP, tag="psmm")
                for kc in range(2):
                    nc.tensor.matmul(
                        ps2[:, 0:NG],
                        w3T_sb[:, kc, F : F + 2],
                        h2_sb[:, kc, 0:NG],
                        start=(kc == 0),
                        stop=False,
                    )
                nc.tensor.matmul(
                    ps2[:, 0:NG],
                    b3r_sb[0:1, F : F + 2],
                    ones_sb[0:1, 0:NG],
                    start=False,
                    stop=True,
                )
                nc.scalar.activation(o2_all[0:2, os_], ps2[:, 0:NG], _ACT.Copy)

            def front(group):
                gn = len(group)
                t0 = group[0]
                xT_sb = gpool.tile([128, 5, 512], BF, tag="xT")
                s2ps = []
                for t in group:
                    stage_a(t)
                for pi in range(0, gn, 2):
                    pts = group[pi : pi + 2]
                    s2p = wpool.tile([128, 2 * H], FP, tag="s2p")
                    for half, t in enumerate(pts):
                        stage_b(t)
                        stage_c(t, xT_sb, t - t0)
                        stage_d1(t, s2p, half)
                        st[t].clear()
                    s2ps.append((s2p, pi))
                vrs = [(stage_d2(s2p), pi) for s2p, pi in s2ps]
                for vr2, pi in vrs:
                    stage_d3(vr2, xT_sb, pi)
                return xT_sb

            def back(xT_sb, group):
                gn = len(group)
                t0 = group[0]
                stage_e(xT_sb, gn, t0)
                os_ = slice(t0 * 128, t0 * 128 + gn * 128)
                for m in range(2):
                    nc.sync.dma_start(
                        out_d[:, m * PAD_ATOMS + t0 * 128 :
                              m * PAD_ATOMS + t0 * 128 + gn * 128],
                        out_all[:, m, os_],
                    )
                nc.sync.dma_start(out2_d[:, os_], o2_all[:, os_])

            # group-level software pipeline: the MLP of group g is emitted
            # after the front (segment sums / avf) of group g+1, so the PE's
            # MLP work overlaps the vector engine's next-group work
            groups = [list(range(g, min(g + GRP, NT))) for g in range(0, NT, GRP)]
            pending = None
            for group in groups:
                xT_sb = front(group)
                if pending is not None:
                    back(*pending)
                pending = (xT_sb, group)
            back(*pending)

    nc.compile()
    return nc


def _prep(atomic_embedding, partial_charges, pair_indices, gs, gv, agh,
          W_gs, W1, b1, W2, b2, W3, b3):
    E = np.ascontiguousarray(np.asarray(atomic_embedding, dtype=np.float32))
    q = np.asarray(partial_charges, dtype=np.float32).reshape(N_ATOMS)
    idx = np.asarray(pair_indices)[1].astype(np.int64)
    n_pairs = idx.shape[0]
    gs = np.asarray(gs, dtype=np.float32)
    gv = np.asarray(gv, dtype=np.float32).reshape(n_pairs, 3 * G)

    order = np.argsort(idx, kind="stable")
    idx_s = idx[order]
    gs_s = gs[order]
    pay_all = np.zeros((n_pairs, PW), dtype=np.float32)
    pay_all[:, 0:G] = gs_s
    pay_all[:, 32:48] = gs_s * q[idx_s][:, None]
    pay_all[:, 64:112] = gv[order]

    # window boundaries: core k, window w covers atoms [k*APC + w*WA, ...)
    bounds = np.zeros((N_CORES, NW + 1), dtype=np.int64)
    counts = np.zeros((N_CORES, NW), dtype=np.int64)
    for k in range(N_CORES):
        for w in range(NW):
            lo = k * APC + w * WA
            hi = min(k * APC + (w + 1) * WA, (k + 1) * APC)
            bounds[k, w] = np.searchsorted(idx_s, lo)
            if w == NW - 1:
                bounds[k, NW] = np.searchsorted(idx_s, hi)
        counts[k] = np.diff(bounds[k])
    budgets = tuple(
        int(max(1, -(-int(counts[:, w].max()) // 128))) for w in range(NW)
    )
    t_total = sum(budgets)

    # shared params (h-major AGH so the avf product/reduce are contiguous)
    aghr = np.asarray(agh, dtype=np.float32).transpose(0, 2, 1).reshape(F, H * G).astype(NP_BF)
    wgsT16 = np.asarray(W_gs, dtype=np.float32).T.astype(NP_BF)
    wgsT = np.zeros((48, F), dtype=NP_BF)
    wgsT[0:G] = wgsT16
    wgsT[32:48] = wgsT16
    W1 = np.asarray(W1, dtype=np.float32)
    # permute MLP input features: [radial_emb, radial_q, vector_emb], drop vector_q
    W1p = np.concatenate([W1[:, 0:F], W1[:, F + H : 2 * F + H], W1[:, F : F + H]], axis=1)
    w1T = np.zeros((640, HID), dtype=NP_BF)
    w1T[0 : 2 * F + H] = W1p.T.astype(NP_BF)
    w2T = np.asarray(W2, dtype=np.float32).T.astype(NP_BF)
    w3T = np.asarray(W3, dtype=np.float32).T.astype(NP_BF)
    b1v = np.asarray(b1, dtype=np.float32).reshape(HID, 1)
    b2v = np.asarray(b2, dtype=np.float32).reshape(HID, 1)
    b3v = np.asarray(b3, dtype=np.float32).reshape(OUT_F, 1)
    ident = np.eye(128, dtype=np.float32)
    arange_wa = np.arange(WA, dtype=np.int64)

    in_maps = []
    for k in range(N_CORES):
        pay = np.zeros((t_total * 128, PW), dtype=np.float32)
        ohm = np.zeros((t_total * 128, WA), dtype=NP_F8)
        off = 0
        for w in range(NW):
            lo_p, hi_p = bounds[k, w], bounds[k, w + 1]
            cnt = hi_p - lo_p
            r0 = off * 128
            pay[r0 : r0 + cnt] = pay_all[lo_p:hi_p]
            loc = (idx_s[lo_p:hi_p] - (k * APC + w * WA)).astype(np.int64)
            ohm[r0 : r0 + cnt] = (loc[:, None] == arange_wa[None, :]).astype(NP_F8)
            off += budgets[w]
        # partition-major device layouts: [128, t_total, width]
        pay_dev = np.ascontiguousarray(
            pay.reshape(t_total, 128, PW).transpose(1, 0, 2)
        )
        ohm_dev = np.ascontiguousarray(
            ohm.reshape(t_total, 128, WA).transpose(1, 0, 2)
        )
        e_k = np.zeros((PAD_ATOMS, F), dtype=np.float32)
        e_k[0:APC] = E[k * APC : (k + 1) * APC]
        in_maps.append(
            {
                "payload": pay_dev.astype(NP_BF),
                "onehot": ohm_dev,
                "eT": np.ascontiguousarray(e_k.T).astype(NP_BF),
                "aghr": aghr,
                "wgsT": wgsT,
                "w1T": w1T,
                "b1": b1v,
                "w2T": w2T,
                "b2": b2v,
                "w3T": w3T,
                "b3r": b3v.reshape(1, OUT_F).astype(NP_BF),
                "ident": ident,
            }
        )
    return budgets, in_maps


def _run(inputs, trace=False):
    budgets, in_maps = _prep(**inputs)
    if budgets not in _cache:
        _cache[budgets] = _build(list(budgets))
    nc = _cache[budgets]
    res = run_bass_kernel_spmd(
        nc, in_maps, core_ids=list(range(N_CORES)), trace=trace
    )
    outs = []
    for k in range(N_CORES):
        o = res.results[k]["out"].reshape(128, 2, PAD_ATOMS)
        o = o.transpose(1, 0, 2).reshape(2 * 128, PAD_ATOMS)
        outs.append(np.concatenate([o, res.results[k]["out2"]], axis=0))
    full = np.concatenate([o[:, :APC] for o in outs], axis=1).T
    full = np.ascontiguousarray(full, dtype=np.float32)
    delta_q = full[:, 0:1]
    f_out = full[:, 1:2]
    delta_a = full[:, 2:]
    return (delta_a, delta_q, f_out), res


def kernel(**inputs):
    out, _ = _run(inputs, trace=False)
    return out
